# revision 37
# baseline (speedup 1.0000x reference)
"""Trainium2 Bass kernel for nn_BioPlausibleSNN.

Design (SNN_MODE "fast", build_fast):
  - Wall time on this axon environment is dominated by host->device input
    shipping (~85 MB/s through the tunnel) plus device exec. So:
      * x is shipped SHARDED: each core gets its 32-batch-row slice as
        xt [D, T*BL] (8 MB/core, 64 MB total) instead of the full 64 MB
        replicated (537 MB total).
      * each core computes its drive shard  din = W_in @ x_k + bias
        ([H, T*BL], bias folded, t=0 uses the no-homeo-fold bias), then ONE
        device AllGather (~16.8 MB/core, ~30 ms) replicates the full-batch
        drive to every core.
  - The membrane/spike scan stays FULL-BATCH-REPLICATED on every core
    (exact homeostasis rates, zero per-step communication; per-step
    collectives cost ~7.4 ms each here and SWDGE remote DMA crashes).
  - Scan recurrent matmul is W-stationary: out[h,b] accumulated over
    h'-chunks with stationary W_rec.T chunks and the h-major spike tile as
    the moving operand. Output lands h-major directly -> the 8 per-step PE
    transposes + 2 scalar copies of the old design are gone, and the
    serial chain is matmul -> add -> spike.
  - State layout h-major: [128 partitions = h%128, cols = (h//128)*B + b].
  - Refractory eliminated algebraically: can_{t+1} = 1 - spk_t - spk_{t-1}.
  - Homeostasis rate applied at the next step's consume phase via the
    Act-engine bias port: mpf[:,c] = Identity(mp[:,c] + bias(-KP*rs[:,c])).
  - Elementwise work spread across DVE (mem/spk/u/syn_pre/can),
    Act (om, mpf), Pool (mp, s1).
  - Spikes stream to DRAM (only this core's 32-batch slice); STDP/elig
    becomes a post-scan dense-matmul phase against precomputed causal
    filters, followed by one output projection (same as the baseline).
"""

import os
import sys

sys.path.insert(0, "/opt/trn_rl_repo")

import numpy as np

import concourse.bass as bass
import concourse.bacc as bacc
import concourse.tile as tile
import concourse.mybir as mybir
from concourse import bass_utils
import concourse.tile_utils as _tile_utils
_tile_utils.max_sbuf_usage = 206 * 1024

F32 = mybir.dt.float32
AX = mybir.AxisListType
OP = mybir.AluOpType
AF = mybir.ActivationFunctionType

# Problem constants
B, T_FULL, D, H, O = 256, 256, 256, 512, 128
NCORES = 8
BL = B // NCORES  # 32 batch rows per core
HC = H // 128     # 4 h-chunks

# fp32 decay constants, exactly as the reference computes them
# (jnp.exp on a float32 scalar).
def _f32exp(x):
    return np.float32(np.exp(np.float64(np.float32(x))))

DM = _f32exp(-1.0 / 20.0)     # mem decay    0x3F7383C6
DS = _f32exp(-1.0 / 8.0)      # syn decay    0x3F61EB51
DPRE = _f32exp(-1.0 / 20.0)
DPOST = _f32exp(-1.0 / 20.0)
DELIG = _f32exp(-1.0 / 100.0)
assert DM.tobytes().hex() == "c683733f" and DS.tobytes().hex() == "51eb613f"
assert DELIG.tobytes().hex() == "e8737d3f"

A_PLUS, A_MINUS = 0.01, 0.012
V_TH = 1.0
HOMEO_TARGET = 8.0 * (1.0 / 1000.0)
HOMEO_REG = 0.002
KP = float(DM) * HOMEO_REG / B   # per-sum rate coefficient applied at t+1


_FILTER_CACHE = {}


def _blob_layout(T):
    """Row layout of the packed small-weights blob ([rows,128] f32). The axon
    tunnel charges ~75 ms per transferred buffer, so everything except xt
    ships as ONE tensor."""
    ents = [("wrt", H * H), ("wit", D * H), ("wot", H * O),
            ("lpre", T * T), ("lpost", T * T), ("clin", T * 128),
            ("bout", BL * O), ("biaspack", 128 * 128),
            ("pidrow", 128), ("xt", D * T * BL)]
    offs, off = {}, 0
    for name, n in ents:
        assert n % 128 == 0
        offs[name] = off // 128
        off += n
    return offs, off // 128


def _host_filters(T):
    """Precompute causal filter matrices for the deferred STDP path (float64)."""
    if T in _FILTER_CACHE:
        return _FILTER_CACHE[T]
    t = np.arange(T)
    # w_e[tau] = sum_{t>=tau} delig^(t-tau)
    we = (1.0 - np.float64(DELIG) ** (T - t)) / (1.0 - np.float64(DELIG))
    s = t[:, None]
    tp = t[None, :]
    causal = (s <= tp).astype(np.float64)
    lpre = causal * (0.1 * A_PLUS) * we[None, :] * np.float64(DPRE) ** np.maximum(tp - s, 0)
    lpost = causal * np.float64(DPOST) ** np.maximum(tp - s, 0)
    # linear spike term: 1 - 0.1*A_MINUS*sum_{t'>=s} we[t']*dpost^(t'-s)
    cpost = (causal * we[None, :] * np.float64(DPOST) ** np.maximum(tp - s, 0)).sum(1)
    clin = 1.0 - 0.1 * A_MINUS * cpost
    out = (lpre.astype(np.float32), lpost.astype(np.float32),
           clin.astype(np.float32))
    _FILTER_CACHE[T] = out
    return out


def build_fast(T=T_FULL, rec_mode="f32", inject=True, cut=None, wide=False,
               pipe=False, p2ov=False):
    """Sharded-input / drive-gather / replicated-scan build."""
    BF16 = mybir.dt.bfloat16
    F32R = mybir.dt.float32r
    # spk2: spike copy in the matmul dtype (Act-produced, satisfies the
    # BIR verifier's "rounded" requirement for reduced-precision matmuls)
    spk2_dt = {"bf16": BF16, "bf16x2": BF16, "f32r": F32R}.get(rec_mode)
    BLOC = B            # all 256 batch rows of state on every core
    W = HC * BLOC       # state tile free width (c-major: c*BLOC + b) = 1024
    TBLK = min(8, T)
    assert T % TBLK == 0
    NBLK = T // TBLK
    SCH = (T + 127) // 128
    TBL = T * BL        # drive-shard columns

    nc = bacc.Bacc("TRN2", target_bir_lowering=False, debug=False,
                   enable_asserts=False, num_devices=NCORES,
                   enable_partition_id=False)

    need_f32_w = rec_mode in ("f32", "f32r")
    _offs, _BR = _blob_layout(T)
    blob = nc.dram_tensor("blob", [_BR, 128], F32, kind="ExternalInput").ap()
    xt = blob[_offs["xt"]:_offs["xt"] + D * TBL // 128, :].rearrange(
        "(d t) q -> d (t q)", d=D)

    def bsl(name, row_extra, p, c):
        r0 = _offs[name] + row_extra
        return blob[r0:r0 + p * c // 128, :].rearrange(
            "(p k) q -> p (k q)", p=p)

    wrt_hi = (nc.dram_tensor("wrt_hi", [H, H], BF16, kind="ExternalInput").ap()
              if not need_f32_w else None)
    wrt_lo = (nc.dram_tensor("wrt_lo", [H, H], BF16, kind="ExternalInput").ap()
              if rec_mode == "bf16x2" else None)
    ident_d = (nc.dram_tensor("ident", [128, 128], F32, kind="ExternalInput").ap()
               if inject else None)
    out_d = nc.dram_tensor("out", [BL, O], F32, kind="ExternalOutput").ap()

    NF = 128 * 128
    import contextlib
    with tile.TileContext(nc) as tc:
        with contextlib.ExitStack() as _st:
            constp = _st.enter_context(tc.tile_pool(name="const", bufs=1))
            drivep = _st.enter_context(tc.tile_pool(name="drive", bufs=2))
            statep = _st.enter_context(tc.tile_pool(name="state", bufs=2))
            spkp = _st.enter_context(tc.tile_pool(name="spk", bufs=2))
            smallp = _st.enter_context(tc.tile_pool(name="small", bufs=4))
            dramp = _st.enter_context(tc.tile_pool(name="dram", bufs=2, space="DRAM"))
            p2p = _st.enter_context(tc.tile_pool(name="p2", bufs=2))
            pre_ps = contextlib.ExitStack()
            psd = pre_ps.enter_context(tc.tile_pool(name="ps_d", bufs=4, space="PSUM"))
            xstage = contextlib.ExitStack()
            xtp = xstage.enter_context(tc.tile_pool(name="xtst", bufs=3))
            dsbp = xstage.enter_context(tc.tile_pool(name="dsb", bufs=3))

            # ---- constants into SBUF --------------------------------------
            wrt_sb, wit_sb, wot_sb = [], [], []
            wrt_hi_sb, wrt_lo_sb, wrt_r_sb = [], [], []
            if rec_mode == "f32r":
                for c in range(HC):
                    tw = dsbp.tile([128, H], F32, tag="wrtst",
                                   name=f"wrtst{c}")
                    nc.sync.dma_start(tw[:, :], bsl("wrt", c * 512, 128, H))
                    tr = constp.tile([128, H], F32R, tag=f"wrtr{c}")
                    nc.scalar.activation(tr[:, :], tw[:, :], AF.Copy)
                    wrt_r_sb.append(tr)
            elif spk2_dt is not None:
                for c in range(HC):
                    th = constp.tile([128, H], BF16, tag=f"wrth{c}")
                    nc.sync.dma_start(th[:, :], wrt_hi[c * 128:(c + 1) * 128, :])
                    wrt_hi_sb.append(th)
                    if rec_mode == "bf16x2":
                        tl_ = constp.tile([128, H], BF16, tag=f"wrtl{c}")
                        nc.sync.dma_start(tl_[:, :],
                                          wrt_lo[c * 128:(c + 1) * 128, :])
                        wrt_lo_sb.append(tl_)
            else:
                for c in range(HC):
                    tw = constp.tile([128, H], F32, tag=f"wrt{c}")
                    nc.sync.dma_start(tw[:, :], bsl("wrt", c * 512, 128, H))
                    wrt_sb.append(tw)
            for k in range(D // 128):
                twi = constp.tile([128, H], F32, tag=f"wit{k}")
                nc.sync.dma_start(twi[:, :], bsl("wit", k * 512, 128, H))
                wit_sb.append(twi)
            for c in range(HC):
                two = constp.tile([128, O], F32, tag=f"wot{c}")
                nc.sync.dma_start(two[:, :], bsl("wot", c * 128, 128, O))
                wot_sb.append(two)
            biaspk_sb = constp.tile([128, 128], F32, tag="biaspk")
            nc.sync.dma_start(biaspk_sb[:, :], bsl("biaspack", 0, 128, 128))
            bias_sb = biaspk_sb[:, 0:HC]
            bias0_sb = biaspk_sb[:, HC:2 * HC]
            bout_sb = constp.tile([BL, O], F32, tag="bout")
            nc.sync.dma_start(bout_sb[:, :], bsl("bout", 0, BL, O))
            zeros_sb = constp.tile([128, W], F32, tag="zeros")
            nc.vector.memset(zeros_sb[:, :], 0.0)
            zeros4_sb = constp.tile([128, HC], F32, tag="zeros4")
            nc.vector.memset(zeros4_sb[:, :], 0.0)
            can0_sb = constp.tile([128, W], F32, tag="can0")
            nc.vector.memset(can0_sb[:, :], 1.0)
            ones_col = constp.tile([128, 1], F32, tag="ones_col")
            nc.vector.memset(ones_col[:, :], 1.0)
            ones_sb = constp.tile([128, 128], F32, tag="ones")
            nc.vector.memset(ones_sb[:, :], 1.0)
            if inject:
                ident_sb = constp.tile([128, 128], F32, tag="ident")
                nc.sync.dma_start(ident_sb[:, :], ident_d[:, :])

            hist = dramp.tile([T, 128, 128], F32, tag="hist")
            g_dram = dramp.tile([128, 128], F32, tag="gdram")
            din = dramp.tile([H, TBL], F32, tag="din")
            dg = dramp.tile([NCORES * H, TBL], F32, tag="dg",
                            addr_space="Shared")

            _pid_reg = nc.sync.alloc_register("pid_from_blob")
            nc.sync.reg_load(_pid_reg,
                             blob[_offs["pidrow"]:_offs["pidrow"] + 1,
                                  0:1].bitcast(mybir.dt.uint32))
            pid_sp = nc.sync.snap(_pid_reg, donate=True, min_val=0,
                                  max_val=NCORES - 1)

            # ---- phase 0: drive shard + AllGather -------------------------
            NJ = TBL // 512
            for j in range(NJ):
                xjs = []
                for k in range(D // 128):
                    xj = xtp.tile([128, 512], F32, tag=f"xj{k}",
                                  name=f"xj{k}_{j}")
                    nc.sync.dma_start(
                        xj[:, :], xt[k * 128:(k + 1) * 128,
                                     j * 512:(j + 1) * 512])
                    xjs.append(xj)
                for c in range(HC):
                    ps = psd.tile([128, 512], F32, tag="psd")
                    for k in range(D // 128):
                        nc.tensor.matmul(
                            ps[:, :],
                            wit_sb[k][:, c * 128:(c + 1) * 128],
                            xjs[k][:, :],
                            start=(k == 0), stop=(k == D // 128 - 1))
                    dsb = dsbp.tile([128, 512], F32, tag="dsb")
                    if j == 0:
                        # cols 0..BL-1 are t=0: bias without the homeo fold
                        nc.scalar.activation(
                            dsb[:, 0:BL], ps[:, 0:BL], AF.Identity,
                            bias=biaspk_sb[:, HC + c:HC + c + 1], scale=1.0)
                        nc.scalar.activation(
                            dsb[:, BL:512], ps[:, BL:512], AF.Identity,
                            bias=biaspk_sb[:, c:c + 1], scale=1.0)
                    else:
                        nc.scalar.activation(
                            dsb[:, :], ps[:, :], AF.Identity,
                            bias=biaspk_sb[:, c:c + 1], scale=1.0)
                    nc.sync.dma_start(
                        din[c * 128:(c + 1) * 128, j * 512:(j + 1) * 512],
                        dsb[:, :])
            nc.gpsimd.collective_compute(
                "AllGather", OP.bypass, replica_groups=[list(range(NCORES))],
                ins=[din.opt()], outs=[dg.opt()])
            xstage.close()   # release the 64KB/partition x staging
            pre_ps.close()

            scan_ps = contextlib.ExitStack()
            psb = scan_ps.enter_context(tc.tile_pool(name="ps_b", bufs=2, space="PSUM"))

            # ---- drive block loads from the gathered tensor ---------------
            drive_tiles = [None] * NBLK

            _dma_engines = [nc.sync, nc.scalar, nc.gpsimd]

            def load_drive_block(bi):
                dt_ = drivep.tile([128, TBLK * W], F32, tag="drive")
                drive_tiles[bi] = dt_
                d4 = dt_[:, :].rearrange("p (t c b) -> p t c b", c=HC, b=BLOC)
                for k in range(NCORES):
                    for c in range(HC):
                        src = dg[k * H + c * 128:k * H + (c + 1) * 128,
                                 bi * TBLK * BL:(bi + 1) * TBLK * BL]
                        eng = _dma_engines[(k * HC + c) % 3]
                        eng.dma_start(
                            d4[:, :, c, k * BL:(k + 1) * BL],
                            src.rearrange("p (t b) -> p t b", b=BL))

            load_drive_block(0)

            d3_of = lambda bi: drive_tiles[bi][:, :].rearrange(
                "p (t q) -> p t q", q=W)

            # ---- phase2 g=0 overlap setup ---------------------------------
            lpre_sb, lpost_sb, clin_sb = [], [], []

            def load_p2_consts():
                for i in range(SCH):
                    r0, r1 = i * 128, min(T, (i + 1) * 128)
                    pw = r1 - r0
                    tl = constp.tile([pw, T], F32, tag=f"lpre{i}",
                                     name=f"lpre{i}")
                    nc.sync.dma_start(tl[:, :], bsl("lpre", r0 * T // 128,
                                                    pw, T))
                    lpre_sb.append(tl)
                    tl2 = constp.tile([pw, T], F32, tag=f"lpost{i}",
                                      name=f"lpost{i}")
                    nc.sync.dma_start(tl2[:, :], bsl("lpost", r0 * T // 128,
                                                     pw, T))
                    lpost_sb.append(tl2)
                    tc_ = constp.tile([pw, 128], F32, tag=f"clin{i}",
                                      name=f"clin{i}")
                    nc.sync.dma_start(tc_[:, :], bsl("clin", r0, pw, 128))
                    clin_sb.append(tc_)

            NT = 32
            FW = NF // NT
            g0_dram = None
            if p2ov and T > 128:
                load_p2_consts()
                g0_dram = dramp.tile([NT, FW], F32, tag="g0d")
                ov_ps = scan_ps.enter_context(
                    tc.tile_pool(name="ps_ov", bufs=1, space="PSUM"))
                ovg_ps = scan_ps.enter_context(
                    tc.tile_pool(name="ps_ovg", bufs=2, space="PSUM"))
                ovp = _st.enter_context(tc.tile_pool(name="ovp", bufs=1))
                hist2v = hist[:, :, :].rearrange("t p f -> t (p f)")

            def emit_g0_block(j):
                """Phase-2 work for j that depends only on hist rows 0..127:
                g0 = ones^T (lpre0@S0 * lpost0@S0) + clin0^T S0, row 0 to DRAM."""
                s0 = ovp.tile([128, FW], F32, tag="ovs0", name=f"ovs0_{j}")
                nc.sync.dma_start(s0[:, :], hist2v[0:128, j * FW:(j + 1) * FW])
                p_ps = ov_ps.tile([128, FW], F32, tag="ovp", name=f"ovp_{j}")
                q_ps = ov_ps.tile([128, FW], F32, tag="ovq", name=f"ovq_{j}")
                nc.tensor.matmul(p_ps[:, :], lpre_sb[0][:, 0:128], s0[:, :],
                                 start=True, stop=True)
                nc.tensor.matmul(q_ps[:, :], lpost_sb[0][:, 0:128], s0[:, :],
                                 start=True, stop=True)
                p_sb = ovp.tile([128, FW], F32, tag="ovpsb", name=f"ovpsb_{j}")
                nc.scalar.activation(p_sb[:, :], p_ps[:, :], AF.Copy)
                y_sb = ovp.tile([128, FW], F32, tag="ovy", name=f"ovy_{j}")
                nc.vector.tensor_tensor(y_sb[:, :], q_ps[:, :], p_sb[:, :],
                                        OP.mult)
                g_ps = ovg_ps.tile([128, FW], F32, tag="ovg", name=f"ovg_{j}")
                nc.tensor.matmul(g_ps[:, :], ones_sb[:, :], y_sb[:, :],
                                 start=True, stop=False, skip_group_check=True)
                nc.tensor.matmul(g_ps[:, :], clin_sb[0][:, :], s0[:, :],
                                 start=False, stop=True, skip_group_check=True)
                g_row = ovp.tile([1, FW], F32, tag="ovrow", name=f"ovrow_{j}")
                nc.scalar.activation(g_row[:, :], g_ps[0:1, :], AF.Copy)
                nc.sync.dma_start(g0_dram[j:j + 1, :], g_row[0:1, :])

            # ---- scan -----------------------------------------------------
            # Recurrences (homeo target const folded into drive bias, rate
            # applied via mpfd at the next step's consume phase; drive and
            # recurrent input meet in PSUM):
            #   mem_t  = s1_t + (dv_t + rec_t)_psum
            #   spk_t  = (mem_t >= 1) * can_t     [+ accum -> batch rate sums]
            #   om_t   = 1 - spk_t;  mp_t = mem_t*om_t
            #   can_{t+1} = om_t - spk_{t-1}
            #   x1_t   = mem_t - mpfd_{t-1}
            #   [step t+1] mpfd_t = dm*mp_t - dm*KP*rs_t   (Act scale+bias)
            #              s1_{t+1} = ds*x1_t + mpfd_t
            KPD = float(np.float32(-KP * float(DM)))
            spk_prev = zeros_sb
            spkb_prev = None
            if spk2_dt is not None:
                zerosb_sb = constp.tile([128, W], spk2_dt, tag="zerosb")
                if spk2_dt == F32R:
                    nc.scalar.activation(zerosb_sb[:, :], zeros_sb[:, :],
                                         AF.Copy)
                else:
                    nc.vector.memset(zerosb_sb[:, :], 0.0)
                spkb_prev = zerosb_sb
            mp_prev = zeros_sb
            rs_prev = zeros4_sb
            can_cur = can0_sb
            x1_prev = None
            s1 = None

            for t in range(T):
                bi = t // TBLK
                if cut == "nodrv":
                    bi = 0
                elif t % TBLK == 0 and bi + 1 < NBLK and drive_tiles[bi + 1] is None:
                    load_drive_block(bi + 1)

                mpfd = zeros_sb
                if t >= 1 and cut not in ("noew", "skel"):
                    rsn = smallp.tile([128, HC], F32, tag="rsn")
                    nc.vector.tensor_scalar(rsn[:, :], rs_prev[:, :],
                                            KPD, 0.0, OP.mult, OP.add)
                    mpfd = statep.tile([128, W], F32, tag="mpfd")
                    for c in range(HC):
                        cs = slice(c * BLOC, (c + 1) * BLOC)
                        nc.scalar.activation(
                            mpfd[:, cs], mp_prev[:, cs], AF.Identity,
                            bias=rsn[:, c:c + 1], scale=float(DM))
                    s1 = statep.tile([128, W], F32, tag="s1")
                    nc.vector.scalar_tensor_tensor(
                        s1[:, :], x1_prev[:, :], float(DS), mpfd[:, :],
                        OP.mult, OP.add)

                # ---- PSUM: drive injected via identity matmul (no spike
                # dependency -> runs while DVE computes this step's spikes),
                # then the W-stationary recurrent matmul accumulates on top;
                # c-outer so the PE can start on spike chunk 0 ----
                if wide:
                    ps_wide = psb.tile([128, W], F32, tag="psw",
                                       name=f"psw_{t}")
                    ps_cur = [ps_wide[:, co * BLOC:(co + 1) * BLOC]
                              for co in range(HC)]
                else:
                    ps_cur = [psb.tile([128, BLOC], F32, tag=f"ps{co}",
                                       name=f"psc{co}_{t}")[:, :]
                              for co in range(HC)]
                dvt = d3_of(bi)[:, t % TBLK, :]
                if inject:
                    for co in range(HC):
                        # PSUM start zeroes the whole 2KB bank: in wide mode
                        # only the first inject per bank may set it
                        st = (co % 2 == 0) if wide else True
                        nc.tensor.matmul(
                            ps_cur[co], ident_sb[:, :],
                            dvt[:, co * BLOC:(co + 1) * BLOC],
                            start=st, stop=False, skip_group_check=True)
                if cut in ("norec", "skel"):
                    pass
                elif rec_mode == "f32r":
                    for c in range(HC):
                        for co in range(HC):
                            nc.tensor.matmul(
                                ps_cur[co],
                                wrt_r_sb[c][:, co * 128:(co + 1) * 128],
                                spkb_prev[:, c * BLOC:(c + 1) * BLOC],
                                start=(not inject and c == 0),
                                stop=(c == HC - 1),
                                skip_group_check=True)
                elif rec_mode == "bf16":
                    for c in range(HC):
                        for co in range(HC):
                            nc.tensor.matmul(
                                ps_cur[co],
                                wrt_hi_sb[c][:, co * 128:(co + 1) * 128],
                                spkb_prev[:, c * BLOC:(c + 1) * BLOC],
                                start=(not inject and c == 0),
                                stop=(c == HC - 1),
                                skip_group_check=True)
                elif rec_mode == "bf16x2":
                    for c in range(HC):
                        for co in range(HC):
                            nc.tensor.matmul(
                                ps_cur[co],
                                wrt_hi_sb[c][:, co * 128:(co + 1) * 128],
                                spkb_prev[:, c * BLOC:(c + 1) * BLOC],
                                start=(not inject and c == 0), stop=False,
                                skip_group_check=True)
                            nc.tensor.matmul(
                                ps_cur[co],
                                wrt_lo_sb[c][:, co * 128:(co + 1) * 128],
                                spkb_prev[:, c * BLOC:(c + 1) * BLOC],
                                start=False, stop=(c == HC - 1),
                                skip_group_check=True)
                elif pipe:
                    # co-outer: each psum region finishes after its own 4
                    # matmuls so the DVE mem/spike halves can pipeline
                    # against the PE tail (inject already zeroed the banks)
                    for co in range(HC):
                        for c in range(HC):
                            nc.tensor.matmul(
                                ps_cur[co],
                                wrt_sb[c][:, co * 128:(co + 1) * 128],
                                spk_prev[:, c * BLOC:(c + 1) * BLOC],
                                start=False,
                                stop=(c == HC - 1),
                                skip_group_check=True)
                else:
                    for c in range(HC):
                        for co in range(HC):
                            st = (not inject and c == 0
                                  and (co % 2 == 0 if wide else True))
                            nc.tensor.matmul(
                                ps_cur[co],
                                wrt_sb[c][:, co * 128:(co + 1) * 128],
                                spk_prev[:, c * BLOC:(c + 1) * BLOC],
                                start=st,
                                stop=(c == HC - 1),
                                skip_group_check=True)

                if inject:
                    s1_ap = zeros_sb[:, :] if s1 is None else s1[:, :]
                    s1_ap = s1_ap
                else:
                    s1dv = statep.tile([128, W], F32, tag="s1dv")
                    if s1 is None:
                        nc.vector.tensor_copy(s1dv[:, :], dvt)
                    else:
                        nc.vector.tensor_tensor(s1dv[:, :], s1[:, :], dvt,
                                                OP.add)
                    s1_ap = s1dv[:, :]
                mem = statep.tile([128, W], F32, tag="mem")
                spk = spkp.tile([128, W], F32, tag="spk")
                rs_new = smallp.tile([128, HC], F32, tag="rsum")
                for co in range(HC):
                    cs = slice(co * BLOC, (co + 1) * BLOC)
                    nc.vector.tensor_tensor(mem[:, cs], s1_ap[:, cs],
                                            ps_cur[co][:, :], OP.add)
                    nc.vector.scalar_tensor_tensor(
                        spk[:, cs], mem[:, cs], 1.0, can_cur[:, cs],
                        OP.is_ge, OP.mult, accum_out=rs_new[:, co:co + 1])
                    if spk2_dt is not None:
                        if co == 0:
                            spkb = spkp.tile([128, W], spk2_dt, tag="spkb")
                        nc.scalar.activation(spkb[:, cs], spk[:, cs], AF.Copy)
                # store only this core's 32-row slice of the spikes
                if cut != "nohist":
                    spk3 = spk[:, :].rearrange("p (c b) -> p c b", b=BLOC)
                    nc.sync.dma_start(
                        hist[t, :, :].rearrange("p (c b) -> p c b", b=BL),
                        spk3[:, :, bass.ts(pid_sp, BL)])

                if cut in ("noew", "skel"):
                    spk_prev = spk
                    if spk2_dt is not None:
                        spkb_prev = spkb
                    continue
                om = spkp.tile([128, W], F32, tag="om")
                nc.scalar.activation(om[:, :], spk[:, :], AF.Identity,
                                     bias=ones_col[:, :], scale=-1.0)
                mp = statep.tile([128, W], F32, tag="mp")
                nc.gpsimd.tensor_tensor(mp[:, :], mem[:, :], om[:, :], OP.mult)
                can_next = spkp.tile([128, W], F32, tag="can")
                nc.gpsimd.tensor_tensor(can_next[:, :], om[:, :],
                                        spk_prev[:, :], OP.subtract)

                if t < T - 1:
                    rs_prev = rs_new
                    x1 = statep.tile([128, W], F32, tag="x1")
                    nc.vector.tensor_tensor(x1[:, :], mem[:, :],
                                            mpfd[:, :], OP.subtract)
                    x1_prev = x1

                spk_prev, mp_prev, can_cur = spk, mp, can_next
                if spk2_dt is not None:
                    spkb_prev = spkb
                if g0_dram is not None and 129 <= t < 129 + NT:
                    emit_g0_block(t - 129)

            # ---- phase 2: deferred STDP/eligibility + output ---------------
            scan_ps.close()
            if cut == "nop2":
                dummy = p2p.tile([BL, O], F32, tag="outsb")
                nc.vector.memset(dummy[:, :], 0.0)
                nc.sync.dma_start(out_d[:, :], dummy[:, :])
            _skip_p2 = (cut == "nop2")
            psp2 = _st.enter_context(tc.tile_pool(name="ps_p2", bufs=2, space="PSUM"))
            if not lpre_sb:
                load_p2_consts()
            g0_done = g0_dram is not None
            hist2 = hist[:, :, :].rearrange("t p f -> t (p f)")
            for j in range(NT if not _skip_p2 else 0):
                s_tiles = []
                for i in range(SCH):
                    r0, r1 = i * 128, min(T, (i + 1) * 128)
                    st = p2p.tile([r1 - r0, FW], F32, tag=f"s{i}")
                    nc.sync.dma_start(st[:, :], hist2[r0:r1, j * FW:(j + 1) * FW])
                    s_tiles.append(st)
                g_ps = psp2.tile([128, FW], F32, tag="gps")
                first_g = True
                for g in range(1 if g0_done else 0, SCH):
                    r0, r1 = g * 128, min(T, (g + 1) * 128)
                    pw = r1 - r0
                    p_ps = psp2.tile([pw, FW], F32, tag="pps")
                    q_ps = psp2.tile([pw, FW], F32, tag="qps")
                    for i in range(g + 1):
                        nc.tensor.matmul(
                            p_ps[:, :], lpre_sb[i][:, r0:r1], s_tiles[i][:, :],
                            start=(i == 0), stop=(i == g))
                        nc.tensor.matmul(
                            q_ps[:, :], lpost_sb[i][:, r0:r1], s_tiles[i][:, :],
                            start=(i == 0), stop=(i == g))
                    p_sb = p2p.tile([pw, FW], F32, tag="psb2")
                    nc.scalar.activation(p_sb[:, :], p_ps[:, :], AF.Copy)
                    y_sb = p2p.tile([pw, FW], F32, tag="ysb")
                    nc.vector.tensor_tensor(y_sb[:, :], q_ps[:, :], p_sb[:, :],
                                            OP.mult)
                    nc.tensor.matmul(g_ps[:, :], ones_sb[:pw, :], y_sb[:, :],
                                     start=first_g, stop=False,
                                     skip_group_check=True)
                    first_g = False
                for i in range(1 if g0_done else 0, SCH):
                    r0, r1 = i * 128, min(T, (i + 1) * 128)
                    nc.tensor.matmul(g_ps[:, :], clin_sb[i][:, :], s_tiles[i][:, :],
                                     start=False, stop=(i == SCH - 1),
                                     skip_group_check=True)
                g_row = p2p.tile([1, FW], F32, tag="grow")
                if g0_done:
                    g0l = ovp.tile([1, FW], F32, tag="g0l", name=f"g0l_{j}")
                    nc.sync.dma_start(g0l[0:1, :], g0_dram[j:j + 1, :])
                    nc.vector.tensor_tensor(g_row[:, :], g_ps[0:1, :],
                                            g0l[0:1, :], OP.add)
                else:
                    nc.scalar.activation(g_row[:, :], g_ps[0:1, :], AF.Copy)
                g_flat = g_dram[:, :].rearrange("p q -> (p q)")
                nc.sync.dma_start(
                    g_flat[j * FW:(j + 1) * FW].unsqueeze(0), g_row[0:1, :])

            if not _skip_p2:
                g_h = p2p.tile([128, 128], F32, tag="gh")
                nc.sync.dma_start(g_h[:, :], g_dram[:, :])
                out_ps = psp2.tile([BL, O], F32, tag="ops")
                for c in range(HC):
                    nc.tensor.matmul(out_ps[:, :], g_h[:, c * 32:(c + 1) * 32],
                                     wot_sb[c][:, :], start=(c == 0),
                                     stop=(c == HC - 1))
                out_sb = p2p.tile([BL, O], F32, tag="outsb")
                nc.vector.scalar_tensor_tensor(
                    out_sb[:, :], out_ps[:, :], 1.0 / T, bout_sb[:, :],
                    OP.mult, OP.add)
                nc.sync.dma_start(out_d[:, :], out_sb[:, :])

    nc.compile()
    return nc


# ---------------------------------------------------------------------------
# Host glue
# ---------------------------------------------------------------------------

def make_inputs(x, W_in, b_in, W_rec, b_rec, W_out, b_out, T):
    """Build the 8 per-core input dicts (numpy only; staging/sharding)."""
    x = np.asarray(x, np.float32)
    lpre, lpost, clin = _host_filters(T)
    clin_rep = np.repeat(clin[:, None], 128, axis=1).astype(np.float32)

    bias = (np.asarray(b_in, np.float32) + np.asarray(b_rec, np.float32))
    biasN = bias + np.float32(float(DM) * HOMEO_REG * HOMEO_TARGET * (1.0 - float(DS)))
    bias0_g = np.ascontiguousarray(bias.reshape(HC, 128).T)
    bias_g = np.ascontiguousarray(biasN.reshape(HC, 128).T)

    wrt = np.ascontiguousarray(np.asarray(W_rec, np.float32).T)
    rec_mode = os.environ.get("SNN_REC", "f32")
    if rec_mode in ("bf16", "bf16x2"):
        import ml_dtypes
        wrt_hi = wrt.astype(ml_dtypes.bfloat16)
        wrt_lo = (wrt - wrt_hi.astype(np.float32)).astype(ml_dtypes.bfloat16)
    else:
        wrt_hi = wrt_lo = wrt
    wit = np.ascontiguousarray(np.asarray(W_in, np.float32).T)
    wot = np.ascontiguousarray(np.asarray(W_out, np.float32).T)
    bout_rep = np.repeat(np.asarray(b_out, np.float32)[None, :], BL, axis=0)
    bout_rep = np.ascontiguousarray(bout_rep)

    ident = np.eye(128, dtype=np.float32)
    offs, BR = _blob_layout(T)
    wtop = offs["xt"] * 128          # weights prefix length in floats
    wpart = np.zeros((wtop,), np.float32)

    def _put(name, arr):
        a = np.ascontiguousarray(arr, dtype=np.float32).reshape(-1)
        wpart[offs[name] * 128: offs[name] * 128 + a.size] = a

    _put("wrt", wrt)
    _put("wit", wit)
    _put("wot", wot)
    _put("lpre", lpre)
    _put("lpost", lpost)
    _put("clin", clin_rep)
    _put("bout", bout_rep)
    biaspack = np.zeros((128, 128), np.float32)
    biaspack[:, 0:HC] = bias_g
    biaspack[:, HC:2 * HC] = bias0_g
    _put("biaspack", biaspack)
    shared = dict(wrt_hi=wrt_hi, wrt_lo=wrt_lo, ident=ident)

    def slice_k(k):
        bk = np.empty((BR * 128,), np.float32)
        bk[:wtop] = wpart
        bk[offs["pidrow"] * 128] = np.array([k], np.uint32).view(np.float32)[0]
        xk = x[k * BL:(k + 1) * BL, :T, :]            # [BL, T, D]
        dest = bk[wtop:].reshape(D, T * BL)
        np.copyto(dest, xk.transpose(2, 1, 0).reshape(D, T * BL))
        return bk.reshape(BR, 128)

    from concurrent.futures import ThreadPoolExecutor
    with ThreadPoolExecutor(max_workers=8) as ex:
        blobs = list(ex.map(slice_k, range(NCORES)))

    in_maps = []
    for k in range(NCORES):
        m = dict(shared)
        m["blob"] = blobs[k]
        in_maps.append(m)
    return in_maps


_CACHE = {}


def kernel(x, W_in, b_in, W_rec, b_rec, W_out, b_out):
    T = x.shape[1]
    rec_mode = os.environ.get("SNN_REC", "f32")
    inject = os.environ.get("SNN_INJECT", "0") == "1"
    wide = os.environ.get("SNN_WIDE", "1") == "1"
    key = ("mod", T, rec_mode, inject, wide)
    if key not in _CACHE:
        _CACHE[key] = build_fast(T=T, rec_mode=rec_mode, inject=inject,
                                 wide=wide)
    nc = _CACHE[key]
    in_maps = make_inputs(x, W_in, b_in, W_rec, b_rec, W_out, b_out, T)
    res = bass_utils.run_bass_kernel_spmd(nc, in_maps, core_ids=list(range(NCORES)))
    outs = [res.results[k]["out"] for k in range(NCORES)]
    full = np.concatenate(outs, axis=0).astype(np.float32)
    return full
